# revision 12
# baseline (speedup 1.0000x reference)
"""CosArcLoss on 8 TRN2 NeuronCores (Bass/Tile).

Math (reference, f32):
    t_i   = preds[i, labels[i]]
    theta = arccos(clip(t_i, -1+1e-12, 1-1e-12))
    num_i = 30*(cos(theta + 0.5) - 0.35)
    S_i   = sum_j exp(30*preds[i,j])
    den_i = exp(num_i) + S_i - exp(30*t_i)
    loss  = mean_i( log(den_i) - num_i )

Everything except S_i is O(B) work and happens on the host in f64 during the
unshard step. The device program is a pure streaming exp(30x) + row-sum:
batch-parallel, 256 rows/core, viewed as [128 partitions, 64000] so each
partition streams two full rows back-to-back. No gather, no collective.

The host ships preds as bf16 (halves HBM traffic; perturbs the final loss
by ~2e-5 relative — the f64 epilogue subtracts exactly exp(30*bf16(t)) so
the target correction is consistent). With bf16 input the kernel is
ScalarE-bound: ACT runs exp at 1 elem/cycle/lane @1.2 GHz ((N+352)/1.2 ns)
regardless of dtype, ~53us for the 64000 elem/lane, while the DMA stream
needs only ~39us at ~428 GB/s. The schedule therefore minimizes instruction
count (NT=7 tiles) and ramps tile sizes geometrically so ScalarE starts
~11us in and never waits again: each tile's exp outpaces the next tile's
arrival. Per-tile accum_out gives the row partial sums; tile boundaries
never straddle the mid-partition row boundary (col 32000), so each accum
slot belongs to exactly one of the partition's two rows; the host combines
slots 0..NT_A-1 -> local row 2p, the rest -> row 2p+1, in f64. The early
accum slots are flushed to DRAM mid-stream from the (otherwise idle) SyncE
queue — emitted after all x dispatches so the in-order engine can't stall
them — and the last slots go out via a ScalarE-dispatched DMA that lands
right after its own final accumulator read.
"""
import numpy as np
from contextlib import ExitStack

import concourse.bass as bass
import concourse.tile as tile
from concourse import bacc, mybir
from concourse.bass_utils import run_bass_kernel_spmd

B, V = 2048, 32000
N_CORES = 8
RPC = B // N_CORES            # 256 rows per core
P = 128                       # SBUF partitions
W = RPC * V // P              # 64000 cols per partition (= 2 rows)

# column tiling of the per-partition stream; boundaries avoid col 32000.
# small leading tiles let ScalarE start ~9us into the stream instead of
# waiting for a huge first tile; the shrinking tail keeps the final exp
# after the last byte short. 6000 cols = 24KB packets (full DMA rate).
TILES = [500, 3000, 6000, 9000, 13500,                           # local row 2p
         18500, 13500]                                           # local row 2p+1
assert sum(TILES[:5]) == V and sum(TILES[5:]) == V
NT = len(TILES)
NT_A = 5                      # tiles 0..NT_A-1 lie in the first row
NFLUSH = 5                    # slots 0..4 flushed early; rest at end

SCALE = 30.0

F32 = mybir.dt.float32
BF16 = mybir.dt.bfloat16
AF = mybir.ActivationFunctionType

_cache = {}


def _build():
    nc = bacc.Bacc("TRN2", target_bir_lowering=False, debug=False,
                   num_devices=N_CORES)
    x = nc.dram_tensor("x", [P, W], BF16, kind="ExternalInput")
    out = nc.dram_tensor("out", [P, NT], F32, kind="ExternalOutput")

    with tile.TileContext(nc) as tc, ExitStack() as ctx:
        xpool = ctx.enter_context(tc.tile_pool(name="x", bufs=4))
        epool = ctx.enter_context(tc.tile_pool(name="e", bufs=1))
        spool = ctx.enter_context(tc.tile_pool(name="s", bufs=1))

        # two accum tiles so the early out-DMA only depends on slots 0..NFLUSH-1
        ssumA = spool.tile([P, NFLUSH], F32)
        ssumB = spool.tile([P, NT - NFLUSH], F32)

        off = 0
        for t, tc_ in enumerate(TILES):
            xt = xpool.tile([P, tc_], BF16, tag="xt")
            nc.sync.dma_start(xt[:], x[:, off:off + tc_])
            et = epool.tile([P, tc_], BF16, tag="et")
            acc = (ssumA[:, t:t + 1] if t < NFLUSH
                   else ssumB[:, t - NFLUSH:t - NFLUSH + 1])
            nc.scalar.activation(
                et[:], xt[:], AF.Exp, scale=SCALE, accum_out=acc,
            )
            off += tc_

        # emitted after every x-tile dispatch so the scheduler cannot park a
        # semaphore-blocked out-DMA in front of x dispatches on in-order
        # SyncE. The final (tiny) flush goes on ScalarE: it lands right
        # after its own last accum read with no cross-engine semaphore hop.
        nc.sync.dma_start(out[:, 0:NFLUSH], ssumA[:])
        nc.scalar.dma_start(out[:, NFLUSH:NT], ssumB[:])

    nc.compile()
    return nc


def _get_nc():
    if "nc" not in _cache:
        _cache["nc"] = _build()
    return _cache["nc"]


def _shard(preds, labels=None):
    """Core c gets rows [c*256, (c+1)*256) as bf16 viewed [128, 64000]."""
    import ml_dtypes
    pb = np.ascontiguousarray(preds, dtype=np.float32).astype(ml_dtypes.bfloat16)
    return [{"x": pb[c * RPC:(c + 1) * RPC].reshape(P, W)}
            for c in range(N_CORES)]


def kernel(preds, labels):
    preds = np.ascontiguousarray(np.asarray(preds), dtype=np.float32)
    labels = np.asarray(labels).astype(np.int64)
    nc = _get_nc()
    res = run_bass_kernel_spmd(nc, _shard(preds), list(range(N_CORES)))

    # unshard: per-row exp-sums S_i, combined in f64
    S = np.empty(B, np.float64)
    p = np.arange(P)
    for c in range(N_CORES):
        o = np.asarray(res.results[c]["out"], np.float64)   # [P, NT]
        S[c * RPC + 2 * p] = o[:, :NT_A].sum(axis=1)        # local rows 2p
        S[c * RPC + 2 * p + 1] = o[:, NT_A:].sum(axis=1)    # local rows 2p+1

    # host epilogue (f64, O(B)): numerator + target correction + mean.
    # The numerator uses the exact f32 target; the correction subtracts
    # exactly what the device summed for the target column: exp(30*bf16(t)).
    import ml_dtypes
    t = preds[np.arange(B), labels]
    tb = t.astype(ml_dtypes.bfloat16).astype(np.float64)
    t = t.astype(np.float64)
    eps = 1e-12
    theta = np.arccos(np.clip(t, -1.0 + eps, 1.0 - eps))
    theta = np.clip(theta, eps, np.pi - eps)
    num = SCALE * (np.cos(theta + 0.5) - 0.35)
    den = np.exp(num) + S - np.exp(SCALE * tb)
    loss = -(num - np.log(den)).mean()
    return np.array(loss, dtype=np.float32)
